# revision 4
# baseline (speedup 1.0000x reference)
"""Trainium2 Bass kernel for nn_BiLSTMCell (graph-LSTM cell).

Math (per batch row):
    g_pre[g] = x @ Wx[g].T + hidden @ Wh[g].T + neighbors @ Wn[g].T + b[g]
    i, f, o = sigmoid(g_pre[0..2]);  s = tanh(g_pre[3])
    next_cell = f * cell + i * s
    next_hidden = o * tanh(next_cell)

Strategy: data-parallel over the batch (8192 -> 1024 rows/core on 8 cores),
weights replicated. On-host we fold x/hidden/neighbors/bias into ONE
augmented operand A = [x | hidden | neighbors | 1] of K = 2053 (zero-padded
to 17*128 = 2176) so each gate pre-activation is a single matmul chain:
    g_pre[g]^T = W_all[g] @ A^T        (computed as 17 accumulating PE
                                        matmuls of [128k,128h]^T @ [128k,512b])
Matmuls run in float32r (FP22 truncation, full PE rate at N=512). The
elementwise combine runs on ScalarE (sigmoid/tanh, one table set) and
VectorE (mul/add), fully overlapped with the PE stream.

Outputs are produced transposed/tiled and unscrambled on the host.
"""

import os
import sys

import numpy as np


def _import_concourse():
    try:
        import concourse.bass  # noqa: F401
        return
    except ImportError:
        pass
    for p in ("/opt/trn_rl_repo", "/root/.axon_site/_ro/trn_rl_repo"):
        if os.path.isdir(p) and p not in sys.path:
            sys.path.insert(0, p)
    import concourse.bass  # noqa: F401


B, IN, H, NB, G = 8192, 1024, 1024, 4, 4
NCORES = 8
BS = B // NCORES        # 1024 batch rows per core
KT = 17                 # k-tiles of 128 (IN+H+NB+1 = 2053 -> 2176)
KAUG = IN + H + NB + 1  # 2053
KPAD = KT * 128         # 2176
HT = H // 128           # 8 h-tiles of 128
BT = BS // 512          # 2 b-tiles of 512


def _split_excess_waits(nc, max_waits=1, drain_max=0):
    """This walrus build's codegen supports very few sync-wait commands per
    instruction (1 for most ops, 0 spare on Drain). Hoist excess sem-waits
    onto preceding wait-only NoOps on the same engine (AND-semantics over
    monotone semaphores makes sequential waiting equivalent)."""
    from concourse import mybir

    uid = [0]
    n_split = 0
    for fn in nc.m.functions:
        for bb in fn.blocks:
            new_insts = []
            for inst in bb.instructions:
                limit = drain_max if type(inst).__name__ == "InstDrain" else max_waits
                si = inst.sync_info
                waits = list(si.on_wait) if si and si.on_wait else []
                if len(waits) > limit:
                    n_split += 1
                    if limit > 0:
                        excess, keep = waits[:-limit], waits[-limit:]
                    else:
                        excess, keep = waits, []
                    for i in range(0, len(excess), max_waits):
                        chunk = excess[i:i + max_waits]
                        nop = mybir.InstNoOp(
                            name=f"waitsplit_{uid[0]}",
                            sync_info=mybir.SyncInfo(on_wait=chunk, on_update=[]),
                        )
                        uid[0] += 1
                        nop.engine = inst.engine
                        new_insts.append(nop)
                    si.on_wait = keep
                    inst.sync_info = si
                new_insts.append(inst)
            bb.instructions = new_insts
    return n_split


_PROG = None


def _build_program():
    import concourse.bass as bass
    import concourse.tile as tile
    from concourse import mybir

    f32 = mybir.dt.float32
    f32r = mybir.dt.float32r
    ACT = mybir.ActivationFunctionType

    nc = bass.Bass()
    at_d = nc.dram_tensor("AT", [128, KT, BS], f32r, kind="ExternalInput")
    w_d = nc.dram_tensor("W", [HT, 128, KT, G * 128], f32r, kind="ExternalInput")
    ct_d = nc.dram_tensor("CT", [HT, BT, 128, 512], f32, kind="ExternalInput")
    ho_d = nc.dram_tensor("hT", [HT, BT, 128, 512], f32, kind="ExternalOutput")
    co_d = nc.dram_tensor("cT", [HT, BT, 128, 512], f32, kind="ExternalOutput")

    with tile.TileContext(nc) as tc:
        with (
            tc.tile_pool(name="at", bufs=1) as p_at,
            tc.tile_pool(name="w", bufs=2) as p_w,
            tc.tile_pool(name="cell", bufs=3) as p_cell,
            tc.tile_pool(name="eps", bufs=2) as p_eps,
            tc.tile_pool(name="outs", bufs=2) as p_out,
            tc.tile_pool(name="ps", bufs=8, space="PSUM") as p_ps,
        ):
            at = p_at.tile([128, KT, BS], f32r, name="at")
            for kk in range(KT):
                nc.sync.dma_start(at[:, kk, :], at_d[:, kk, :])

            for hh in range(HT):
                wt = p_w.tile([128, KT, G * 128], f32r, name="wt", tag="wt")
                nc.sync.dma_start(wt[:], w_d[hh])
                for bb in range(BT):
                    ct = p_cell.tile([128, 512], f32, name="ct", tag="ct")
                    nc.sync.dma_start(ct[:], ct_d[hh, bb])

                    ps = []
                    for g in range(G):
                        pt = p_ps.tile([128, 512], f32, name=f"pt{g}", tag="ps")
                        for kk in range(KT):
                            nc.tensor.matmul(
                                pt[:],
                                wt[:, kk, g * 128:(g + 1) * 128],
                                at[:, kk, bb * 512:(bb + 1) * 512],
                                start=(kk == 0),
                                stop=(kk == KT - 1),
                            )
                        ps.append(pt)

                    sig_i = p_eps.tile([128, 512], f32, name="sig_i", tag="sig_i")
                    nc.scalar.activation(sig_i[:], ps[0][:], ACT.Sigmoid)
                    sig_f = p_eps.tile([128, 512], f32, name="sig_f", tag="sig_f")
                    nc.scalar.activation(sig_f[:], ps[1][:], ACT.Sigmoid)
                    sig_o = p_eps.tile([128, 512], f32, name="sig_o", tag="sig_o")
                    nc.scalar.activation(sig_o[:], ps[2][:], ACT.Sigmoid)
                    tan_s = p_eps.tile([128, 512], f32, name="tan_s", tag="tan_s")
                    nc.scalar.activation(tan_s[:], ps[3][:], ACT.Tanh)

                    t_is = p_eps.tile([128, 512], f32, name="t_is", tag="t_is")
                    nc.vector.tensor_mul(t_is[:], sig_i[:], tan_s[:])
                    t_fc = p_eps.tile([128, 512], f32, name="t_fc", tag="t_fc")
                    nc.vector.tensor_mul(t_fc[:], sig_f[:], ct[:])
                    c_new = p_out.tile([128, 512], f32, name="c_new", tag="c_new")
                    nc.vector.tensor_add(c_new[:], t_is[:], t_fc[:])
                    tan_c = p_eps.tile([128, 512], f32, name="tan_c", tag="tan_c")
                    nc.scalar.activation(tan_c[:], c_new[:], ACT.Tanh)
                    h_new = p_out.tile([128, 512], f32, name="h_new", tag="h_new")
                    nc.vector.tensor_mul(h_new[:], sig_o[:], tan_c[:])

                    nc.gpsimd.dma_start(co_d[hh, bb], c_new[:])
                    nc.gpsimd.dma_start(ho_d[hh, bb], h_new[:])

    _split_excess_waits(nc)
    return nc


def _get_program():
    global _PROG
    if _PROG is None:
        _PROG = _build_program()
    return _PROG


def _prep_inputs(x, hidden, cell, neighbors, Wx, Wh, Wn, b):
    """Host-side shard/relayout. Returns per-core input maps."""
    x = np.asarray(x, np.float32)
    hidden = np.asarray(hidden, np.float32)
    cell = np.asarray(cell, np.float32)
    neighbors = np.asarray(neighbors, np.float32)
    Wx = np.asarray(Wx, np.float32)
    Wh = np.asarray(Wh, np.float32)
    Wn = np.asarray(Wn, np.float32)
    b = np.asarray(b, np.float32)

    # Augmented operand A = [x | hidden | neighbors | 1] zero-padded to KPAD.
    A = np.zeros((B, KPAD), np.float32)
    A[:, :IN] = x
    A[:, IN:IN + H] = hidden
    A[:, IN + H:IN + H + NB] = neighbors
    A[:, IN + H + NB] = 1.0

    # W_all[g] = [Wx[g] | Wh[g] | Wn[g] | b[g]] zero-padded to KPAD.
    W_all = np.zeros((G, H, KPAD), np.float32)
    W_all[:, :, :IN] = Wx
    W_all[:, :, IN:IN + H] = Wh
    W_all[:, :, IN + H:IN + H + NB] = Wn
    W_all[:, :, IN + H + NB] = b

    # SBUF weight layout: [hh, p(k), kk, g*128 + j(h)]
    w_host = np.ascontiguousarray(
        W_all.reshape(G, HT, 128, KT, 128).transpose(1, 4, 3, 0, 2)
    ).reshape(HT, 128, KT, G * 128)

    in_maps = []
    for c in range(NCORES):
        sl = slice(c * BS, (c + 1) * BS)
        # A^T tiled: [p(k), kk, b]
        at_host = np.ascontiguousarray(
            A[sl].T.reshape(KT, 128, BS).transpose(1, 0, 2)
        )
        # cell^T tiled: [hh, bb, j(h), n(b)]
        ct_host = np.ascontiguousarray(
            cell[sl].T.reshape(HT, 128, BT, 512).transpose(0, 2, 1, 3)
        )
        in_maps.append({"AT": at_host, "W": w_host, "CT": ct_host})
    return in_maps


def _gather_outputs(results):
    """Invert the per-core [HT, BT, 128, 512] transposed tiling."""
    h_parts, c_parts = [], []
    for c in range(NCORES):
        hT = np.asarray(results[c]["hT"])
        cT = np.asarray(results[c]["cT"])
        # [hh, bb, j, n] -> [hh*128+j, bb*512+n] -> transpose to [b, h]
        h_parts.append(hT.transpose(0, 2, 1, 3).reshape(H, BS).T)
        c_parts.append(cT.transpose(0, 2, 1, 3).reshape(H, BS).T)
    next_hidden = np.ascontiguousarray(np.concatenate(h_parts, axis=0), dtype=np.float32)
    next_cell = np.ascontiguousarray(np.concatenate(c_parts, axis=0), dtype=np.float32)
    return next_hidden, next_cell


def _run(in_maps, trace=False, tmpdir=None):
    _import_concourse()
    from concourse.bass_utils import run_bass_kernel_spmd

    if trace:
        _install_ntff_shim()
    nc = _get_program()
    res = run_bass_kernel_spmd(
        nc, in_maps, list(range(NCORES)), trace=trace, tmpdir=tmpdir
    )
    return res


def _install_ntff_shim():
    """Shim antenv.axon_hooks (absent in this image) so trace=True works."""
    import types

    if "antenv.axon_hooks" not in sys.modules:
        mod = types.ModuleType("antenv.axon_hooks")
        mod._hook = None
        mod.set_axon_ntff_profile_hook = lambda h: setattr(mod, "_hook", h)
        mod.get_axon_ntff_profile_hook = lambda: mod._hook
        sys.modules["antenv.axon_hooks"] = mod
        try:
            import antenv
            antenv.axon_hooks = mod
        except ImportError:
            pass
    mod = sys.modules["antenv.axon_hooks"]
    if mod._hook is None:
        from trn_agent_boot.trn_boot import _ntff_profile_via_ctypes
        mod._hook = _ntff_profile_via_ctypes("/opt/axon/libaxon_pjrt.so")
    from concourse import bass_utils
    bass_utils.upload_artifacts = lambda tmpdir: f"local:{tmpdir}"


def kernel(x, hidden, cell, neighbors, Wx, Wh, Wn, b):
    _import_concourse()
    in_maps = _prep_inputs(x, hidden, cell, neighbors, Wx, Wh, Wn, b)
    res = _run(in_maps, trace=False)
    return _gather_outputs(res.results)


# revision 6
# speedup vs baseline: 1.0600x; 1.0600x over previous
"""Trainium2 Bass kernel for nn_BiLSTMCell (graph-LSTM cell).

Math (per batch row):
    g_pre[g] = x @ Wx[g].T + hidden @ Wh[g].T + neighbors @ Wn[g].T + b[g]
    i, f, o = sigmoid(g_pre[0..2]);  s = tanh(g_pre[3])
    next_cell = f * cell + i * s
    next_hidden = o * tanh(next_cell)

Strategy: data-parallel over the batch (8192 -> 1024 rows/core on 8 cores),
weights replicated. On-host we fold x/hidden/neighbors/bias into ONE
augmented operand A = [x | hidden | neighbors | 1] of K = 2053 (zero-padded
to 17*128 = 2176) so each gate pre-activation is a single matmul chain:
    g_pre[g]^T = W_all[g] @ A^T        (computed as 17 accumulating PE
                                        matmuls of [128k,128h]^T @ [128k,512b])
Matmuls run in float32r (FP22 truncation, full PE rate at N=512). The
elementwise combine runs on ScalarE (sigmoid/tanh, one table set) and
VectorE (mul/add), fully overlapped with the PE stream.

Outputs are produced transposed/tiled and unscrambled on the host.
"""

import os
import sys

import numpy as np


def _import_concourse():
    try:
        import concourse.bass  # noqa: F401
        return
    except ImportError:
        pass
    for p in ("/opt/trn_rl_repo", "/root/.axon_site/_ro/trn_rl_repo"):
        if os.path.isdir(p) and p not in sys.path:
            sys.path.insert(0, p)
    import concourse.bass  # noqa: F401


B, IN, H, NB, G = 8192, 1024, 1024, 4, 4
NCORES = 8
BS = B // NCORES        # 1024 batch rows per core
KT = 17                 # k-tiles of 128 (IN+H+NB+1 = 2053 -> 2176)
KAUG = IN + H + NB + 1  # 2053
KPAD = KT * 128         # 2176
HT = H // 128           # 8 h-tiles of 128
BT = BS // 512          # 2 b-tiles of 512


def _split_excess_waits(nc, max_waits=1, drain_max=0):
    """This walrus build's codegen supports very few sync-wait commands per
    instruction (1 for most ops, 0 spare on Drain). Hoist excess sem-waits
    onto preceding wait-only NoOps on the same engine (AND-semantics over
    monotone semaphores makes sequential waiting equivalent)."""
    from concourse import mybir

    uid = [0]
    n_split = 0
    for fn in nc.m.functions:
        for bb in fn.blocks:
            new_insts = []
            for inst in bb.instructions:
                limit = drain_max if type(inst).__name__ == "InstDrain" else max_waits
                si = inst.sync_info
                waits = list(si.on_wait) if si and si.on_wait else []
                if len(waits) > limit:
                    n_split += 1
                    if limit > 0:
                        excess, keep = waits[:-limit], waits[-limit:]
                    else:
                        excess, keep = waits, []
                    for i in range(0, len(excess), max_waits):
                        chunk = excess[i:i + max_waits]
                        nop = mybir.InstNoOp(
                            name=f"waitsplit_{uid[0]}",
                            sync_info=mybir.SyncInfo(on_wait=chunk, on_update=[]),
                        )
                        uid[0] += 1
                        nop.engine = inst.engine
                        new_insts.append(nop)
                    si.on_wait = keep
                    inst.sync_info = si
                new_insts.append(inst)
            bb.instructions = new_insts
    return n_split


_PROG = None


def _build_program():
    import concourse.bass as bass
    import concourse.tile as tile
    from concourse import mybir

    f32 = mybir.dt.float32
    f32r = mybir.dt.float32r
    ACT = mybir.ActivationFunctionType

    nc = bass.Bass()
    at_d = nc.dram_tensor("AT", [128, KT, BS], f32r, kind="ExternalInput")
    w_d = nc.dram_tensor("W", [HT, 128, KT, G * 128], f32r, kind="ExternalInput")
    ct_d = nc.dram_tensor("CT", [HT, BT, 128, 512], f32, kind="ExternalInput")
    ho_d = nc.dram_tensor("hT", [HT, BT, 128, 512], f32, kind="ExternalOutput")
    co_d = nc.dram_tensor("cT", [HT, BT, 128, 512], f32, kind="ExternalOutput")

    with tile.TileContext(nc) as tc:
        with (
            tc.tile_pool(name="at", bufs=1) as p_at,
            tc.tile_pool(name="w", bufs=2) as p_w,
            tc.tile_pool(name="cell", bufs=3) as p_cell,
            tc.tile_pool(name="eps", bufs=2) as p_eps,
            tc.tile_pool(name="outs", bufs=2) as p_out,
            tc.tile_pool(name="ps", bufs=8, space="PSUM") as p_ps,
        ):
            at = p_at.tile([128, KT, BS], f32r, name="at")

            for hh in range(HT):
                wt = p_w.tile([128, KT, G * 128], f32r, name="wt", tag="wt")
                if hh == 0:
                    # Interleave W0/AT chunk loads per k-tile across three
                    # queues so the first accumulation chain can start after
                    # the first k-chunk lands instead of after the full
                    # 13 MB head.
                    for kk in range(KT):
                        nc.sync.dma_start(wt[:, kk, :], w_d[0, :, kk, :])
                        nc.scalar.dma_start(at[:, kk, 0:512], at_d[:, kk, 0:512])
                        nc.gpsimd.dma_start(at[:, kk, 512:BS], at_d[:, kk, 512:BS])
                else:
                    nc.sync.dma_start(wt[:], w_d[hh])
                for bb in range(BT):
                    ct = p_cell.tile([128, 512], f32, name="ct", tag="ct")
                    nc.sync.dma_start(ct[:], ct_d[hh, bb])

                    ps = []
                    for g in range(G):
                        pt = p_ps.tile([128, 512], f32, name=f"pt{g}", tag="ps")
                        for kk in range(KT):
                            nc.tensor.matmul(
                                pt[:],
                                wt[:, kk, g * 128:(g + 1) * 128],
                                at[:, kk, bb * 512:(bb + 1) * 512],
                                start=(kk == 0),
                                stop=(kk == KT - 1),
                            )
                        ps.append(pt)

                    sig_i = p_eps.tile([128, 512], f32, name="sig_i", tag="sig_i")
                    nc.scalar.activation(sig_i[:], ps[0][:], ACT.Sigmoid)
                    sig_f = p_eps.tile([128, 512], f32, name="sig_f", tag="sig_f")
                    nc.scalar.activation(sig_f[:], ps[1][:], ACT.Sigmoid)
                    sig_o = p_eps.tile([128, 512], f32, name="sig_o", tag="sig_o")
                    nc.scalar.activation(sig_o[:], ps[2][:], ACT.Sigmoid)
                    tan_s = p_eps.tile([128, 512], f32, name="tan_s", tag="tan_s")
                    nc.scalar.activation(tan_s[:], ps[3][:], ACT.Tanh)

                    t_is = p_eps.tile([128, 512], f32, name="t_is", tag="t_is")
                    nc.vector.tensor_mul(t_is[:], sig_i[:], tan_s[:])
                    t_fc = p_eps.tile([128, 512], f32, name="t_fc", tag="t_fc")
                    nc.vector.tensor_mul(t_fc[:], sig_f[:], ct[:])
                    c_new = p_out.tile([128, 512], f32, name="c_new", tag="c_new")
                    nc.vector.tensor_add(c_new[:], t_is[:], t_fc[:])
                    tan_c = p_eps.tile([128, 512], f32, name="tan_c", tag="tan_c")
                    nc.scalar.activation(tan_c[:], c_new[:], ACT.Tanh)
                    h_new = p_out.tile([128, 512], f32, name="h_new", tag="h_new")
                    nc.vector.tensor_mul(h_new[:], sig_o[:], tan_c[:])

                    nc.gpsimd.dma_start(co_d[hh, bb], c_new[:])
                    nc.gpsimd.dma_start(ho_d[hh, bb], h_new[:])

    _split_excess_waits(nc)
    return nc


def _get_program():
    global _PROG
    if _PROG is None:
        _PROG = _build_program()
    return _PROG


def _prep_inputs(x, hidden, cell, neighbors, Wx, Wh, Wn, b):
    """Host-side shard/relayout. Returns per-core input maps."""
    x = np.asarray(x, np.float32)
    hidden = np.asarray(hidden, np.float32)
    cell = np.asarray(cell, np.float32)
    neighbors = np.asarray(neighbors, np.float32)
    Wx = np.asarray(Wx, np.float32)
    Wh = np.asarray(Wh, np.float32)
    Wn = np.asarray(Wn, np.float32)
    b = np.asarray(b, np.float32)

    # Augmented operand A = [x | hidden | neighbors | 1] zero-padded to KPAD.
    A = np.zeros((B, KPAD), np.float32)
    A[:, :IN] = x
    A[:, IN:IN + H] = hidden
    A[:, IN + H:IN + H + NB] = neighbors
    A[:, IN + H + NB] = 1.0

    # W_all[g] = [Wx[g] | Wh[g] | Wn[g] | b[g]] zero-padded to KPAD.
    W_all = np.zeros((G, H, KPAD), np.float32)
    W_all[:, :, :IN] = Wx
    W_all[:, :, IN:IN + H] = Wh
    W_all[:, :, IN + H:IN + H + NB] = Wn
    W_all[:, :, IN + H + NB] = b

    # SBUF weight layout: [hh, p(k), kk, g*128 + j(h)]
    w_host = np.ascontiguousarray(
        W_all.reshape(G, HT, 128, KT, 128).transpose(1, 4, 3, 0, 2)
    ).reshape(HT, 128, KT, G * 128)

    in_maps = []
    for c in range(NCORES):
        sl = slice(c * BS, (c + 1) * BS)
        # A^T tiled: [p(k), kk, b]
        at_host = np.ascontiguousarray(
            A[sl].T.reshape(KT, 128, BS).transpose(1, 0, 2)
        )
        # cell^T tiled: [hh, bb, j(h), n(b)]
        ct_host = np.ascontiguousarray(
            cell[sl].T.reshape(HT, 128, BT, 512).transpose(0, 2, 1, 3)
        )
        in_maps.append({"AT": at_host, "W": w_host, "CT": ct_host})
    return in_maps


def _gather_outputs(results):
    """Invert the per-core [HT, BT, 128, 512] transposed tiling."""
    h_parts, c_parts = [], []
    for c in range(NCORES):
        hT = np.asarray(results[c]["hT"])
        cT = np.asarray(results[c]["cT"])
        # [hh, bb, j, n] -> [hh*128+j, bb*512+n] -> transpose to [b, h]
        h_parts.append(hT.transpose(0, 2, 1, 3).reshape(H, BS).T)
        c_parts.append(cT.transpose(0, 2, 1, 3).reshape(H, BS).T)
    next_hidden = np.ascontiguousarray(np.concatenate(h_parts, axis=0), dtype=np.float32)
    next_cell = np.ascontiguousarray(np.concatenate(c_parts, axis=0), dtype=np.float32)
    return next_hidden, next_cell


def _run(in_maps, trace=False, tmpdir=None):
    _import_concourse()
    from concourse.bass_utils import run_bass_kernel_spmd

    if trace:
        _install_ntff_shim()
    nc = _get_program()
    res = run_bass_kernel_spmd(
        nc, in_maps, list(range(NCORES)), trace=trace, tmpdir=tmpdir
    )
    return res


def _install_ntff_shim():
    """Shim antenv.axon_hooks (absent in this image) so trace=True works."""
    import types

    if "antenv.axon_hooks" not in sys.modules:
        mod = types.ModuleType("antenv.axon_hooks")
        mod._hook = None
        mod.set_axon_ntff_profile_hook = lambda h: setattr(mod, "_hook", h)
        mod.get_axon_ntff_profile_hook = lambda: mod._hook
        sys.modules["antenv.axon_hooks"] = mod
        try:
            import antenv
            antenv.axon_hooks = mod
        except ImportError:
            pass
    mod = sys.modules["antenv.axon_hooks"]
    if mod._hook is None:
        from trn_agent_boot.trn_boot import _ntff_profile_via_ctypes
        mod._hook = _ntff_profile_via_ctypes("/opt/axon/libaxon_pjrt.so")
    from concourse import bass_utils
    bass_utils.upload_artifacts = lambda tmpdir: f"local:{tmpdir}"


def kernel(x, hidden, cell, neighbors, Wx, Wh, Wn, b):
    _import_concourse()
    in_maps = _prep_inputs(x, hidden, cell, neighbors, Wx, Wh, Wn, b)
    res = _run(in_maps, trace=False)
    return _gather_outputs(res.results)
